# revision 29
# baseline (speedup 1.0000x reference)
"""Distance-loss kernel for Trainium2 (8 NeuronCores, SPMD data-parallel).

loss = sum_{b,c,h} || output[b,c,h,:] - target[b,c,h,:] + eps ||_2

Strategy: flatten both (16,8,512,512) f32 inputs to rows of W=512
(B*C*H = 65536 rows), shard rows contiguously across 8 cores (8192
rows/core).  Each core streams its 2 x 16 MiB in 16-row-per-partition
tiles (4 MiB per DMA, double-buffered; last tile tapered into 4 x 1 MiB
so the compute tail is short), computes d = (x + eps) - y on the vector
engine in 8-row chunks and per-row sums of squares on the scalar (ACT)
engine, writing rowsq [128, 64] per core.  The host finishes in
float64: sqrt per row, sum.  Raw hand-scheduled bacc pipeline (no Tile
entry/exit barriers).  Memory-bound: per-core roofline ~= 32 MiB at
~390-425 GB/s ~= 80-90 us; measured per-execution ~66-85 us.
"""

import numpy as np

import concourse.tile as tile
from concourse import bacc, bass_utils, mybir
from contextlib import ExitStack

F32 = mybir.dt.float32
NLANES = 8
NBUF = 3

EPS = 1e-6
N_CORES = 8
B, C, H, W = 16, 8, 512, 512
ROWS = B * C * H  # 65536 total rows of length W
ROWS_PER_CORE = ROWS // N_CORES  # 8192
P = 128  # SBUF partitions
R = 8    # rows packed per partition line (16 KiB contiguous per partition)


def _tile_schedule(rows_per_core: int, taper: bool):
    """List of (row_start, rows_per_partition) tiles covering rows_per_core.

    With taper, the final full tile is split into 4 small tiles so the
    compute tail after the last DMA lands is short.
    """
    tiles = rows_per_core // (P * R)
    assert tiles * P * R == rows_per_core
    sched = [(t * P * R, R) for t in range(tiles)]
    if taper and tiles >= 2:
        base, _ = sched.pop()
        step = R // 4
        for s in range(4):
            sched.append((base + s * P * step, step))
    return sched


def build_bass(rows_per_core: int = ROWS_PER_CORE, bufs: int = 3, loops: int = 1,
               taper: bool = True):
    """Build the per-core SPMD Bass program.

    loops > 1 repeats the streaming body (same data) for timing-by-delta;
    the output is unchanged (the repeats are idempotent).
    """
    sched = _tile_schedule(rows_per_core, taper)
    ncols = sum(r for _, r in sched)

    nc = bacc.Bacc("TRN2", target_bir_lowering=False, debug=False)
    x = nc.dram_tensor("x", [rows_per_core, W], mybir.dt.float32, kind="ExternalInput").ap()
    y = nc.dram_tensor("y", [rows_per_core, W], mybir.dt.float32, kind="ExternalInput").ap()
    out = nc.dram_tensor("out", [P, 1], mybir.dt.float32, kind="ExternalOutput").ap()

    with tile.TileContext(nc) as tc:
        with (
            tc.tile_pool(name="xp", bufs=bufs) as xp,
            tc.tile_pool(name="yp", bufs=bufs) as yp,
            tc.tile_pool(name="dp", bufs=2) as dp,
            tc.tile_pool(name="sq", bufs=2) as sqp,
            tc.tile_pool(name="st", bufs=1) as stp,
        ):
            # per-row sums of squares: one column per (tile, packed-row)
            rowsq = stp.tile([P, ncols * loops], mybir.dt.float32)
            col = 0
            for i in range(len(sched) * loops):
                start, r = sched[i % len(sched)]
                xvt = x[start:start + P * r, :].rearrange("(p r) w -> p r w", p=P, r=r)
                yvt = y[start:start + P * r, :].rearrange("(p r) w -> p r w", p=P, r=r)
                xt = xp.tile([P, R, W], mybir.dt.float32, tag="xt")
                nc.sync.dma_start(xt[:, :r, :], xvt)
                yt = yp.tile([P, R, W], mybir.dt.float32, tag="yt")
                nc.sync.dma_start(yt[:, :r, :], yvt)

                d = dp.tile([P, R, W], mybir.dt.float32, tag="d")
                # d = (x + eps) - y   (one DVE pass over the tile)
                nc.vector.scalar_tensor_tensor(
                    out=d[:, :r, :],
                    in0=xt[:, :r, :],
                    scalar=EPS,
                    in1=yt[:, :r, :],
                    op0=mybir.AluOpType.add,
                    op1=mybir.AluOpType.subtract,
                )
                # per row: sum of squares via ACT Square + free-dim accumulate
                for j in range(r):
                    sq = sqp.tile([P, W], mybir.dt.float32, tag="sq")
                    nc.scalar.activation(
                        out=sq[:],
                        in_=d[:, j, :],
                        func=mybir.ActivationFunctionType.Square,
                        accum_out=rowsq[:, col % (ncols * loops):col % (ncols * loops) + 1],
                    )
                    col += 1

            # row_norm = sqrt(rowsq); per-partition partial = sum(row_norm)
            # (columns beyond ncols stay zero when loops == 1)
            norms = stp.tile([P, ncols * loops], mybir.dt.float32)
            rowsum = stp.tile([P, 1], mybir.dt.float32)
            nc.scalar.activation(
                out=norms[:, :ncols],
                in_=rowsq[:, :ncols],
                func=mybir.ActivationFunctionType.Sqrt,
                accum_out=rowsum[:],
            )
            nc.sync.dma_start(out[:], rowsum[:])
    nc.compile()
    return nc


def build_bass_looped(loops: int):
    return build_bass(loops=loops)


XYBUF = 2
DBUF = 3
DCH = 4  # max rows per subtract chunk (finer -> ACT trails DVE closer)


def sched16(rows_per_core: int, rpp: int = 16, taper: bool = True):
    tiles = rows_per_core // (P * rpp)
    assert tiles * P * rpp == rows_per_core
    sched = [(t * P * rpp, rpp) for t in range(tiles)]
    if taper and tiles >= 2 and rpp >= 8:
        # split the last tile progressively (8,4,2,1,1 rows) so the compute
        # tail after the final DMA is one 1-row chunk (~1.5 us)
        base, _ = sched.pop()
        off = 0
        for step in (rpp // 2, rpp // 4, rpp // 8, rpp // 16, rpp // 16):
            if step == 0:
                continue
            sched.append((base + off * P, step))
            off += step
        assert off == rpp
    return sched


def build_raw(rows_per_core: int = ROWS_PER_CORE, rpp: int = 16, taper: bool = True,
                metaloops: int = 1):
    sched = sched16(rows_per_core, rpp, taper)
    n = len(sched)
    ncols = sum(r for _, r in sched)

    # per-tile subtract chunks: list of (row_off, rows)
    tile_chunks = []
    for _, r in sched:
        chunks = []
        off = 0
        while off < r:
            c = min(DCH, r - off)
            chunks.append((off, c))
            off += c
        tile_chunks.append(chunks)

    # --- static bookkeeping across reps ------------------------------------
    # global sub index s -> dve tick s+1; act ticks count rows squared
    sub_of_tile_last = []   # per global tile g: last sub index
    act_after_sub = []      # per global sub s: act tick after its rows done
    s_idx = 0
    a = 0
    for rep in range(metaloops):
        for i in range(n):
            for (_, c) in tile_chunks[i]:
                a += c
                act_after_sub.append(a)
                s_idx += 1
            sub_of_tile_last.append(s_idx - 1)
    total_subs = s_idx
    total_rows = a

    lane_ticks = [0] * NLANES
    issue_lane = []
    k = 0
    for rep in range(metaloops):
        for i in range(n):
            for _ in range(2):
                lane = k % NLANES
                lane_ticks[lane] += 16
                issue_lane.append((lane, lane_ticks[lane]))
                k += 1
        for _ in range(2):  # out#1 (bulk) and out#2 (tail) stores
            lane = k % NLANES
            lane_ticks[lane] += 16
            issue_lane.append((lane, lane_ticks[lane]))
            k += 1

    nc = bacc.Bacc("TRN2", target_bir_lowering=False, debug=False)
    x = nc.dram_tensor("x", [rows_per_core, W], F32, kind="ExternalInput").ap()
    y = nc.dram_tensor("y", [rows_per_core, W], F32, kind="ExternalInput").ap()
    out = nc.dram_tensor("out", [P, ncols], F32, kind="ExternalOutput").ap()

    def view(ap, start, r):
        return ap[start:start + P * r, :].rearrange("(p r) w -> p (r w)", p=P, r=r)

    with ExitStack() as ctx:
        sb = lambda name, shape: ctx.enter_context(nc.sbuf_tensor(name, shape, F32))
        sem = lambda name: ctx.enter_context(nc.semaphore(name))
        xs = [sb(f"xb{j}", [P, rpp * W]) for j in range(XYBUF)]
        ys = [sb(f"yb{j}", [P, rpp * W]) for j in range(XYBUF)]
        ds = [sb(f"db{j}", [P, DCH * W]) for j in range(DBUF)]
        sqbs = [sb(f"sqb{j}", [P, W]) for j in range(3)]
        rowsq = sb("rowsq", [P, ncols])
        lanes = [sem(f"dma{j}") for j in range(NLANES)]
        dve_sem = sem("dve_sem")
        act_sem = sem("act_sem")
        block = ctx.enter_context(nc.Block())

        @block.sync
        def _(sync):
            dma_idx = 0
            for rep in range(metaloops):
                if rep > 0:
                    # bound cross-rep DMA overlap so each lane has at most
                    # one outstanding transfer (metaloops timing NEFFs only)
                    l2, t2 = issue_lane[rep * (2 * n + 2) - 1]
                    sync.wait_ge(lanes[l2], t2)
                for i, (start, r) in enumerate(sched):
                    g = rep * n + i
                    if g >= XYBUF:
                        # x/y slot g%XYBUF frees when tile g-XYBUF's subs retire
                        sync.wait_ge(dve_sem, sub_of_tile_last[g - XYBUF] + 1)
                    lx, _ = issue_lane[dma_idx]
                    ly, _ = issue_lane[dma_idx + 1]
                    sync.dma_start(xs[g % XYBUF][:, :r * W], view(x, start, r)).then_inc(lanes[lx], 16)
                    sync.dma_start(ys[g % XYBUF][:, :r * W], view(y, start, r)).then_inc(lanes[ly], 16)
                    dma_idx += 2
                dma_idx += 2  # out#1/out#2 are issued from the ACT engine
            # program end: waiting on the two out stores' receipts suffices —
            # out#2's completion proves ACT ran to its end, which transitively
            # proves every DVE subtract and every load DMA completed (each was
            # consumed through a semaphore wait upstream).
            l1, t1 = issue_lane[metaloops * (2 * n + 2) - 2]
            l2, t2 = issue_lane[metaloops * (2 * n + 2) - 1]
            sync.wait_ge(lanes[l1], t1)
            sync.wait_ge(lanes[l2], t2)

        @block.vector
        def _(vector):
            dma_idx = 0
            s = 0
            for rep in range(metaloops):
                for i, (start, r) in enumerate(sched):
                    g = rep * n + i
                    lx, tx = issue_lane[dma_idx]
                    ly, ty = issue_lane[dma_idx + 1]
                    dma_idx += 2
                    vector.wait_ge(lanes[lx], tx)
                    vector.wait_ge(lanes[ly], ty)
                    for (off, c) in tile_chunks[i]:
                        if s >= DBUF:
                            # d slot s%DBUF frees when ACT finished sub s-DBUF
                            vector.wait_ge(act_sem, act_after_sub[s - DBUF])
                        nc.vector.scalar_tensor_tensor(
                            out=ds[s % DBUF][:, :c * W],
                            in0=xs[g % XYBUF][:, off * W:(off + c) * W],
                            scalar=EPS,
                            in1=ys[g % XYBUF][:, off * W:(off + c) * W],
                            op0=mybir.AluOpType.add,
                            op1=mybir.AluOpType.subtract,
                        ).then_inc(dve_sem, 1)
                        s += 1
                dma_idx += 2  # skip the rep's two out stores

        @block.scalar
        def _(scalar):
            # out#1 ships the bulk of rowsq while the taper still streams
            # (ACT ring drains independently of the SP load ring); out#2 is
            # the tiny tail store.  Issued from ACT so program order + a
            # cheap self-wait replaces the cross-engine sem hop.
            SPLIT = n - 3
            col_split = sum(r for _, r in sched[:SPLIT])
            rows_per_rep = total_rows // metaloops
            a = 0
            s = 0
            for rep in range(metaloops):
                if rep > 0:
                    # previous rep's out stores must have read rowsq before
                    # this rep's squares overwrite it (satisfied at issue)
                    l1, t1 = issue_lane[rep * (2 * n + 2) - 2]
                    l2, t2 = issue_lane[rep * (2 * n + 2) - 1]
                    scalar.wait_ge(lanes[l1], t1)
                    scalar.wait_ge(lanes[l2], t2)
                col = 0
                for i, (start, r) in enumerate(sched):
                    for (off, c) in tile_chunks[i]:
                        scalar.wait_ge(dve_sem, s + 1)
                        for j in range(c):
                            if a >= 3:
                                # scratch slot owner (a-3) retired; already
                                # true at issue -> no stall
                                scalar.wait_ge(act_sem, a - 2)
                            nc.scalar.activation(
                                out=sqbs[a % 3][:],
                                in_=ds[s % DBUF][:, j * W:(j + 1) * W],
                                func=mybir.ActivationFunctionType.Square,
                                accum_out=rowsq[:, col:col + 1],
                            ).then_inc(act_sem, 1)
                            a += 1
                            col += 1
                        s += 1
                    if i == SPLIT - 1:
                        # bulk store: cols [0, col_split) are final
                        scalar.wait_ge(act_sem, rep * rows_per_rep + col_split)
                        lo1, _ = issue_lane[(rep + 1) * (2 * n + 2) - 2]
                        scalar.dma_start(
                            out[:, :col_split], rowsq[:, :col_split]
                        ).then_inc(lanes[lo1], 16)
                # tail store: remaining cols
                scalar.wait_ge(act_sem, (rep + 1) * rows_per_rep)
                lo2, _ = issue_lane[(rep + 1) * (2 * n + 2) - 1]
                scalar.dma_start(
                    out[:, col_split:], rowsq[:, col_split:]
                ).then_inc(lanes[lo2], 16)

    nc.compile()
    return nc


_NC_CACHE = {}


def _run_in_subprocess(output: np.ndarray, target: np.ndarray) -> np.ndarray:
    """Last-ditch recovery: a wedged accelerator mesh (NRT_EXEC_UNIT_
    UNRECOVERABLE inherited from a previous process) is only ever cleared
    by a fresh process in practice.  Re-run this kernel there."""
    import os
    import subprocess
    import tempfile

    mydir = os.path.dirname(os.path.abspath(__file__))
    with tempfile.TemporaryDirectory() as td:
        inp = os.path.join(td, "in.npz")
        outp = os.path.join(td, "out.npy")
        np.savez(inp, output=output, target=target)
        code = (
            "import sys, numpy as np\n"
            f"sys.path.insert(0, {mydir!r})\n"
            "import kernel\n"
            f"d = np.load({inp!r})\n"
            "r = kernel.kernel(output=d['output'], target=d['target'])\n"
            f"np.save({outp!r}, np.asarray(r))\n"
        )
        env = dict(os.environ)
        env["KERNEL_NO_SUBPROC"] = "1"  # no recursive respawns
        subprocess.run(["python", "-c", code], check=True, env=env, timeout=1200)
        return np.load(outp)[()]


def kernel(output: np.ndarray, target: np.ndarray) -> np.ndarray:
    assert output.shape == (B, C, H, W) and target.shape == (B, C, H, W)
    if "nc" not in _NC_CACHE:
        _NC_CACHE["nc"] = build_raw()
    nc = _NC_CACHE["nc"]

    X = np.ascontiguousarray(output, dtype=np.float32).reshape(N_CORES, ROWS_PER_CORE, W)
    Y = np.ascontiguousarray(target, dtype=np.float32).reshape(N_CORES, ROWS_PER_CORE, W)
    in_maps = [{"x": X[k], "y": Y[k]} for k in range(N_CORES)]

    # The tunneled device occasionally comes up wedged from a previous
    # process (NRT_EXEC_UNIT_UNRECOVERABLE on the first execution).  In-
    # process backend resets rarely fix a desynced mesh; a fresh process
    # reliably does, so the final fallback re-runs there.
    import os
    last_err = None
    for attempt in range(3):
        try:
            if attempt > 0:
                import time
                time.sleep(5 * attempt)
                try:
                    import jax
                    jax.clear_caches()
                    jax.extend.backend.clear_backends()
                except Exception:
                    pass
            res = bass_utils.run_bass_kernel_spmd(nc, in_maps, core_ids=list(range(N_CORES)))
            break
        except Exception as e:  # noqa: BLE001
            last_err = e
    else:
        if os.environ.get("KERNEL_NO_SUBPROC"):
            raise last_err
        try:
            return _run_in_subprocess(output, target)
        except Exception:
            raise last_err

    # host finish in float64: per-row sum of squares -> norm -> scalar sum
    # (device emits rowsq [P, ncols], one column per row; halves == 1)
    total = 0.0
    for m in res.results:
        rowsq = m["out"].astype(np.float64)
        total += float(np.sqrt(rowsq).sum())
    return np.asarray(total, dtype=np.float32)
